# revision 1
# baseline (speedup 1.0000x reference)
"""ContextRetentionLayer Trainium2 kernel.

Reference computation (per token t, d=1024, W=512 memory slots):
    s[t, w]   = (x[t] . mb[w]) / 32
    attn[t]   = softmax_w(s[t])
    r[t]      = sum_w attn[t, w] * mb[w]
    g[t]      = sigmoid(x[t] @ gw.T + gb)
    out[t]    = g[t] * x[t] + (1 - g[t]) * r[t]

Sharding: 4x4096 = 16384 tokens split evenly across 8 cores (2048 each);
memory_bank / gate weights replicated.

Device-side layout is fully transposed (d on partitions, tokens on the free
axis) so every matmul contracts along the partition dim with no on-chip
transposes:
    sT[w, t]  = sum_d mbT[d, w] * xT[d, t]      (lhsT = mbT chunk)
    den[t]    = sum_w exp(sT)[w, t]             (lhsT = ones column)
    rT[d, t]  = sum_w mb[w, d] * attnT[w, t]    (lhsT = mb chunk, natural)
    zT[e, t]  = sum_d gwT[d, e] * xT[d, t]      (lhsT = gwT chunk)
softmax reduces over w via the ones-matmul; the reciprocal runs on VectorE
(reciprocal_approx_accurate) and is broadcast across partitions on GpSimd.
mbT is derived on-chip by PE-transposing mb (saves a 2MB load on the critical
startup path). Skipping the softmax max-subtraction is safe: scores/32 is
~N(0,1) here, far from fp32 overflow.

Matmul tensors are float32r end-to-end (full PE rate at moving dim >= 256;
plain fp32 matmul is 4 cycles/row). Measured on HW: ~114 us/core steady-state,
~= the 109 us pure-matmul floor; rel err vs fp32 reference ~2e-4.
"""

import numpy as np

import concourse.bass as bass
import concourse.tile as tile
from concourse import bacc, bass_utils, mybir
from concourse.bass import ts

AF = mybir.ActivationFunctionType
F32 = mybir.dt.float32
F32R = mybir.dt.float32r

N_CORES = 8
B, S, D = 4, 4096, 1024
W = 512
T_CORE = (B * S) // N_CORES  # 2048 tokens per core
T_TILE = 512                 # moving free dim per matmul (fp32 max)
NT = T_CORE // T_TILE        # 4 token tiles
DC = D // 128                # 8 chunks of the embed dim
WC = W // 128                # 4 chunks of the memory window


def _body(tc: tile.TileContext, reps: int = 1):
    nc = tc.nc

    xT = nc.dram_tensor("xt", (D, T_CORE), F32R, kind="ExternalInput").ap()
    mb = nc.dram_tensor("mb", (W, D), F32R, kind="ExternalInput").ap()
    gwT = nc.dram_tensor("gwt", (D, D), F32R, kind="ExternalInput").ap()
    gb = nc.dram_tensor("gb", (D,), F32, kind="ExternalInput").ap()
    ones_d = nc.dram_tensor("ones", (128, 1), F32R, kind="ExternalInput").ap()
    ident_d = nc.dram_tensor("ident", (128, 128), F32R, kind="ExternalInput").ap()
    outT = nc.dram_tensor("outt", (D, T_CORE), F32, kind="ExternalOutput").ap()

    for _rep in range(reps):
        _emit_once(tc, xT, mb, gwT, gb, ones_d, ident_d, outT)


def _emit_once(tc, xT, mb, gwT, gb, ones_d, ident_d, outT):
    nc = tc.nc
    with (
        tc.tile_pool(name="const", bufs=1) as const,
        tc.tile_pool(name="big", bufs=1) as big,
        tc.tile_pool(name="work", bufs=3) as work,
        tc.tile_pool(name="mm_ps", bufs=7, space="PSUM") as mm_ps,
        tc.tile_pool(name="den_psp", bufs=1, space="PSUM") as den_psp,
    ):
        # ---- tiles: constants (replicated weights) + resident transposed x.
        # All large loads are chunked and emitted in need-order so the PE's
        # first dependencies (mbT + x tile 0) land first; the pass-2 weights
        # (mb, gwT) stream in behind the remaining x tiles.
        mbT_s = const.tile([128, DC, W], F32R)
        mb_s = const.tile([128, WC, D], F32R)
        gwT_s = const.tile([128, DC, D], F32R)
        gb_s = const.tile([128, DC], F32)
        ones_s = const.tile([128, 1], F32R)
        x_s = big.tile([128, DC, T_CORE], F32R)

        mbv = mb.rearrange("(c p) d -> p c d", p=128)
        gwTv = gwT.rearrange("(c p) e -> p c e", p=128)
        xTv = xT.rearrange("(c p) t -> p c t", p=128)

        def load_x(ti, split=1):
            h = DC // split
            for i in range(split):
                nc.sync.dma_start(
                    out=x_s[:, i * h : (i + 1) * h, ts(ti, T_TILE)],
                    in_=xTv[:, i * h : (i + 1) * h, ts(ti, T_TILE)],
                )

        # need-ordered loads: ident (gates the transposes), then mb (feeds
        # the on-chip transpose for pass 1 AND pass 2's retrieved matmul) and
        # x tile 0; gwT (pass 2 gate) last.
        ident = const.tile([128, 128], F32R)
        nc.sync.dma_start(out=ident, in_=ident_d)
        for wc in range(WC):
            nc.sync.dma_start(out=mb_s[:, wc, :], in_=mbv[:, wc, :])
        load_x(0, split=2)
        nc.sync.dma_start(out=ones_s, in_=ones_d)
        nc.sync.dma_start(out=gb_s, in_=gb.rearrange("(c p) -> p c", p=128))

        # mbT = mb.T via PE transpose (f32r, 1.5 cyc/row); DVE copies the
        # PSUM result out, rounding to f32r.
        for wc in range(WC):
            for dc in range(DC):
                t_ps = mm_ps.tile([128, 128], F32R, tag="mm")
                nc.tensor.transpose(t_ps, mb_s[:, wc, ts(dc, 128)], ident)
                nc.vector.tensor_copy(mbT_s[:, dc, ts(wc, 128)], t_ps)

        load_x(1)
        load_x(2)
        load_x(3)
        for dc in range(DC):
            nc.sync.dma_start(out=gwT_s[:, dc, :], in_=gwTv[:, dc, :])

        at_s = big.tile([128, WC, T_CORE], F32R)  # exp(s/32), then attn in place
        rd_s = big.tile([1, T_CORE], F32)         # 1 / denominator
        rb_s = big.tile([128, T_CORE], F32)       # broadcast across partitions

        # ---- pass 1: scores, exp, denominators, attn normalize
        for ti in range(NT):
            tsl = ts(ti, T_TILE)
            den_ps = den_psp.tile([1, T_TILE], F32, tag="den")
            for wc in range(WC):
                s_ps = mm_ps.tile([128, T_TILE], F32, tag="mm")
                for dc in range(DC):
                    nc.tensor.matmul(
                        s_ps,
                        lhsT=mbT_s[:, dc, ts(wc, 128)],
                        rhs=x_s[:, dc, tsl],
                        start=(dc == 0),
                        stop=(dc == DC - 1),
                    )
                nc.scalar.activation(
                    out=at_s[:, wc, tsl], in_=s_ps, func=AF.Exp, scale=1.0 / 32.0
                )
                nc.tensor.matmul(
                    den_ps,
                    lhsT=ones_s,
                    rhs=at_s[:, wc, tsl],
                    start=(wc == 0),
                    stop=(wc == WC - 1),
                )
            rscr = work.tile([1, T_TILE], F32, tag="rscr")
            nc.vector.reciprocal_approx_accurate(
                out=rd_s[:, tsl], in_=den_ps, scratch=rscr
            )
            nc.gpsimd.partition_broadcast(rb_s[:, tsl], rd_s[:, tsl])
            for wc in range(WC):
                nc.vector.tensor_mul(at_s[:, wc, tsl], at_s[:, wc, tsl], rb_s[:, tsl])

        # ---- pass 2: retrieved, gate, combine. The final (ti, dc) iteration
        # is split into half-width slices so the post-PE combine/store tail is
        # shorter before the kernel drain.
        outv = outT.rearrange("(c p) t -> p c t", p=128)

        def p2_iter(dc, t0, tw):
            tsl = slice(t0, t0 + tw)
            z_ps = mm_ps.tile([128, tw], F32, tag="mm")
            for kc in range(DC):
                nc.tensor.matmul(
                    z_ps,
                    lhsT=gwT_s[:, kc, ts(dc, 128)],
                    rhs=x_s[:, kc, tsl],
                    start=(kc == 0),
                    stop=(kc == DC - 1),
                )
            g = work.tile([128, tw], F32, tag="g")
            nc.scalar.activation(
                out=g, in_=z_ps, func=AF.Sigmoid, bias=gb_s[:, dc : dc + 1]
            )
            r_ps = mm_ps.tile([128, tw], F32, tag="mm")
            for wc in range(WC):
                nc.tensor.matmul(
                    r_ps,
                    lhsT=mb_s[:, wc, ts(dc, 128)],
                    rhs=at_s[:, wc, tsl],
                    start=(wc == 0),
                    stop=(wc == WC - 1),
                )
            o = work.tile([128, tw], F32, tag="o")
            nc.vector.tensor_sub(o, x_s[:, dc, tsl].bitcast(F32), r_ps)
            nc.vector.tensor_mul(o, o, g)
            nc.vector.tensor_add(o, o, r_ps)
            nc.sync.dma_start(out=outv[:, dc, tsl], in_=o)

        for ti in range(NT):
            for dc in range(DC):
                if ti == NT - 1 and dc == DC - 1:
                    p2_iter(dc, ti * T_TILE, T_TILE // 2)
                    p2_iter(dc, ti * T_TILE + T_TILE // 2, T_TILE // 2)
                else:
                    p2_iter(dc, ti * T_TILE, T_TILE)


_NC_CACHE = None


def _build_nc(reps: int = 1):
    global _NC_CACHE
    if reps == 1 and _NC_CACHE is not None:
        return _NC_CACHE
    nc = bacc.Bacc("TRN2", target_bir_lowering=False, debug=False,
                   enable_asserts=False)
    with tile.TileContext(nc) as tc:
        _body(tc, reps)
    nc.compile()
    if reps == 1:
        _NC_CACHE = nc
    return nc


def make_in_maps(x, memory_bank, gate_w, gate_b):
    x = np.ascontiguousarray(np.asarray(x, np.float32)).reshape(B * S, D)
    mb_n = np.ascontiguousarray(np.asarray(memory_bank, np.float32))
    gwT_n = np.ascontiguousarray(np.asarray(gate_w, np.float32).T)
    gb_n = np.ascontiguousarray(np.asarray(gate_b, np.float32))
    in_maps = []
    for c in range(N_CORES):
        xs = x[c * T_CORE : (c + 1) * T_CORE]
        in_maps.append(
            {
                "xt": np.ascontiguousarray(xs.T),
                "mb": mb_n,
                "gwt": gwT_n,
                "gb": gb_n,
                "ones": np.ones((128, 1), np.float32),
                "ident": np.eye(128, dtype=np.float32),
            }
        )
    return in_maps


def assemble_out(results):
    shards = [results[c]["outt"].T for c in range(N_CORES)]
    return np.concatenate(shards, axis=0).reshape(B, S, D).astype(np.float32)


def kernel(x, memory_bank, gate_w, gate_b, _run_kwargs=None):
    nc = _build_nc()
    in_maps = make_in_maps(x, memory_bank, gate_w, gate_b)
    res = bass_utils.run_bass_kernel_spmd(
        nc, in_maps, core_ids=list(range(N_CORES)), **(_run_kwargs or {})
    )
    out = assemble_out(res.results)
    if _run_kwargs:
        kernel.last_result = res
    return out



# revision 5
# speedup vs baseline: 1.1543x; 1.1543x over previous
"""ContextRetentionLayer Trainium2 kernel — fp8 DoubleRow version.

Reference computation (per token t, d=1024, W=512 memory slots):
    s[t, w]   = (x[t] . mb[w]) / 32
    attn[t]   = softmax_w(s[t])
    r[t]      = sum_w attn[t, w] * mb[w]
    g[t]      = sigmoid(x[t] @ gw.T + gb)
    out[t]    = g[t] * x[t] + (1 - g[t]) * r[t]

Sharding: 4x4096 = 16384 tokens split evenly across 8 cores (2048 each);
memory_bank / gate weights replicated.

All three big matmuls (scores, retrieved, gate) run in fp8 e4m3 with
perf_mode=DoubleRow: operands live as [128, KC, free] with K-chunk pairs
contracted 256 at a time at ~2x the bf16/f32r column rate.  Tolerance is
2e-2; CPU sim of this exact quantization measures rel err ~1.8e-2
(gate-path dominated; see sim_fp8.py).

Numerics:
  x8 = e4m3(x); mb8 = e4m3(mb); gw8 = e4m3(32*gw)
  s_ps[w,t] = sum_d mb8T . x8          (PSUM f32, = 32*s approx)
  at8       = e4m3(exp(s_ps/32))
  den_ps    = sum_w at8/64             (ones lhsT = 1/64)
  rb        = bcast(1/den_ps) = 64/den
  at8      *= rb                       (in place -> 64*attn, e4m3; keeps
                                        attn out of e4m3 subnormals)
  r_ps      = sum_w mb8 . at8          (= 64*r)
  z_ps      = sum_d gw8T . x8          (= 32*z approx)
  g         = sigmoid(z_ps/32 + gb)
  out       = bf16( r_ps/64 + g*(xc - r_ps/64) ),  xc = bf16(x)
Combine is 3 fused DVE ops (scalar_tensor_tensor folds the 1/64).

Softmax max-subtraction is skipped: scores/32 ~ N(0,1), exp <= ~110 < 240
(e4m3 max), no overflow.  Weight-stationary loop order (weights reused
across 2-4 token tiles per LDWEIGHTS) since DoubleRow disables FWL.
"""

import numpy as np
import ml_dtypes

import concourse.bass as bass
import concourse.tile as tile
from concourse import bacc, bass_utils, mybir
from concourse.bass import ts

AF = mybir.ActivationFunctionType
ALU = mybir.AluOpType
F32 = mybir.dt.float32
BF16 = mybir.dt.bfloat16
F8E4 = mybir.dt.float8e4
E4NP = ml_dtypes.float8_e4m3
BFNP = ml_dtypes.bfloat16

N_CORES = 8
B, S, D = 4, 4096, 1024
W = 512
T_CORE = (B * S) // N_CORES  # 2048 tokens per core
T_TILE = 512                 # moving free dim per matmul (DR pairs: rhs 1024)
NT = T_CORE // T_TILE        # 4 token tiles
DC = D // 128                # 8 chunks of the embed dim
WC = W // 128                # 4 chunks of the memory window

# "dr": natural [128, KC, M] weights, perf_mode=DoubleRow
# "dri": host-interleaved weights, perf_mode=DoubleRowSwInterleave
PERF = "dr"
GATE_BF16 = False            # fallback: gate matmul in bf16 (better precision)

_PM = {
    "dr": mybir.MatmulPerfMode.DoubleRow,
    "dri": mybir.MatmulPerfMode.DoubleRowSwInterleave,
}


def _body(tc: tile.TileContext, reps: int = 1):
    nc = tc.nc

    x8 = nc.dram_tensor("x8", (D, T_CORE), F8E4, kind="ExternalInput").ap()
    xc = nc.dram_tensor("xc", (D, T_CORE), BF16, kind="ExternalInput").ap()
    gb = nc.dram_tensor("gb", (D,), F32, kind="ExternalInput").ap()
    ones = nc.dram_tensor("ones", (128, 1), F8E4, kind="ExternalInput").ap()
    if PERF == "dr":
        mbt = nc.dram_tensor("mbt", (D, W), F8E4, kind="ExternalInput").ap()
        mbw = nc.dram_tensor("mbw", (W, D), F8E4, kind="ExternalInput").ap()
    else:
        mbt = nc.dram_tensor("mbt", (128, DC // 2, WC, 256), F8E4,
                             kind="ExternalInput").ap()
        mbw = nc.dram_tensor("mbw", (128, WC // 2, DC, 256), F8E4,
                             kind="ExternalInput").ap()
    if GATE_BF16:
        gwt = nc.dram_tensor("gwt", (D, D), BF16, kind="ExternalInput").ap()
    elif PERF == "dr":
        gwt = nc.dram_tensor("gwt", (D, D), F8E4, kind="ExternalInput").ap()
    else:
        gwt = nc.dram_tensor("gwt", (128, DC // 2, DC, 256), F8E4,
                             kind="ExternalInput").ap()
    outT = nc.dram_tensor("outt", (D, T_CORE), BF16, kind="ExternalOutput").ap()

    for _rep in range(reps):
        _emit_once(tc, x8, xc, gb, ones, mbt, mbw, gwt, outT)


def _emit_once(tc, x8, xc, gb, ones, mbt, mbw, gwt, outT):
    nc = tc.nc
    pm = _PM[PERF]
    with (
        tc.tile_pool(name="const", bufs=1) as const,
        tc.tile_pool(name="big", bufs=1) as big,
        tc.tile_pool(name="work", bufs=3) as work,
        tc.tile_pool(name="mm_ps", bufs=7, space="PSUM") as mm_ps,
        tc.tile_pool(name="den_psp", bufs=1, space="PSUM") as den_psp,
    ):
        # ---- SBUF tiles
        if PERF == "dr":
            mbt_s = const.tile([128, DC, W], F8E4)
            mbw_s = const.tile([128, WC, D], F8E4)
        else:
            mbt_s = const.tile([128, DC // 2, WC, 256], F8E4)
            mbw_s = const.tile([128, WC // 2, DC, 256], F8E4)
        if GATE_BF16:
            gwt_s = const.tile([128, DC, D], BF16)
        elif PERF == "dr":
            gwt_s = const.tile([128, DC, D], F8E4)
        else:
            gwt_s = const.tile([128, DC // 2, DC, 256], F8E4)
        gb_s = const.tile([128, DC], F32)
        ones_s = const.tile([128, 1], F8E4)

        x8_s = big.tile([128, DC, T_CORE], F8E4)
        xc_s = big.tile([128, DC, T_CORE], BF16)
        at8_s = big.tile([128, WC, T_CORE], F8E4)
        rd_s = big.tile([1, T_CORE], F32)
        rb_s = big.tile([128, T_CORE], F32)

        x8v = x8.rearrange("(c p) t -> p c t", p=128)
        xcv = xc.rearrange("(c p) t -> p c t", p=128)
        outv = outT.rearrange("(c p) t -> p c t", p=128)

        # ---- need-ordered loads: scores weights + x tile 0 first.
        if PERF == "dr":
            mbtv = mbt.rearrange("(c p) w -> p c w", p=128)
            for c in range(DC):
                nc.sync.dma_start(out=mbt_s[:, c, :], in_=mbtv[:, c, :])
        else:
            nc.sync.dma_start(out=mbt_s, in_=mbt)
        nc.sync.dma_start(out=x8_s[:, :, ts(0, T_TILE)], in_=x8v[:, :, ts(0, T_TILE)])
        nc.sync.dma_start(out=ones_s, in_=ones)
        nc.sync.dma_start(out=gb_s, in_=gb.rearrange("(c p) -> p c", p=128))
        for ti in range(1, NT):
            nc.sync.dma_start(
                out=x8_s[:, :, ts(ti, T_TILE)], in_=x8v[:, :, ts(ti, T_TILE)]
            )
        if PERF == "dr":
            mbwv = mbw.rearrange("(c p) d -> p c d", p=128)
            for c in range(WC):
                nc.sync.dma_start(out=mbw_s[:, c, :], in_=mbwv[:, c, :])
        else:
            nc.sync.dma_start(out=mbw_s, in_=mbw)
        if GATE_BF16 or PERF == "dr":
            gwtv = gwt.rearrange("(c p) e -> p c e", p=128)
            for c in range(DC):
                nc.sync.dma_start(out=gwt_s[:, c, :], in_=gwtv[:, c, :])
        else:
            nc.sync.dma_start(out=gwt_s, in_=gwt)
        for ti in range(NT):
            nc.sync.dma_start(
                out=xc_s[:, :, ts(ti, T_TILE)], in_=xcv[:, :, ts(ti, T_TILE)]
            )

        def score_w(dcp, wc):
            if PERF == "dr":
                return mbt_s[:, 2 * dcp : 2 * dcp + 2, ts(wc, 128)]
            return mbt_s[:, dcp, wc, :]

        def retr_w(wcp, dc):
            if PERF == "dr":
                return mbw_s[:, 2 * wcp : 2 * wcp + 2, ts(dc, 128)]
            return mbw_s[:, wcp, dc, :]

        def gate_w(dcp, ec):
            if PERF == "dr":
                return gwt_s[:, 2 * dcp : 2 * dcp + 2, ts(ec, 128)]
            return gwt_s[:, dcp, ec, :]

        # ---- pass 1: scores + exp.  Weight-stationary: each (wc, dcp)
        # weight pair streams all 4 token tiles before switching.
        for wc in range(WC):
            s_ps = [mm_ps.tile([128, T_TILE], F32, tag="mm", name=f"sps{wc}_{i}")
                    for i in range(NT)]
            for dcp in range(DC // 2):
                for ti in range(NT):
                    nc.tensor.matmul(
                        s_ps[ti],
                        lhsT=score_w(dcp, wc),
                        rhs=x8_s[:, 2 * dcp : 2 * dcp + 2, ts(ti, T_TILE)],
                        start=(dcp == 0),
                        stop=(dcp == DC // 2 - 1),
                        perf_mode=pm,
                    )
            for ti in range(NT):
                nc.scalar.activation(
                    out=at8_s[:, wc, ts(ti, T_TILE)], in_=s_ps[ti],
                    func=AF.Exp, scale=1.0 / 32.0,
                )

        # ---- denominators + normalize (at8 <- 64*attn, in place)
        for ti in range(NT):
            tsl = ts(ti, T_TILE)
            den_ps = den_psp.tile([1, T_TILE], F32, tag="den")
            for wc in range(WC):
                nc.tensor.matmul(
                    den_ps,
                    lhsT=ones_s,
                    rhs=at8_s[:, wc, tsl],
                    start=(wc == 0),
                    stop=(wc == WC - 1),
                )
            rscr = work.tile([1, T_TILE], F32, tag="rscr")
            nc.vector.reciprocal_approx_accurate(
                out=rd_s[:, tsl], in_=den_ps, scratch=rscr
            )
            nc.gpsimd.partition_broadcast(rb_s[:, tsl], rd_s[:, tsl])
            for wc in range(WC):
                nc.vector.tensor_mul(at8_s[:, wc, tsl], at8_s[:, wc, tsl], rb_s[:, tsl])

        # ---- pass 2: gate + retrieved + combine, in token-tile pairs so at
        # most ~4 PSUM banks per (tib, dc) group and weights are reused x2.
        for tib in range(NT // 2):
            tis = (2 * tib, 2 * tib + 1)
            tsls = [ts(t, T_TILE) for t in tis]
            for dc in range(DC):
                z_ps = [mm_ps.tile([128, T_TILE], F32, tag="mm", name=f"zps{j}")
                        for j in range(2)]
                if GATE_BF16:
                    for kc in range(DC):
                        for j in range(2):
                            nc.tensor.matmul(
                                z_ps[j],
                                lhsT=gwt_s[:, kc, ts(dc, 128)],
                                rhs=xc_s[:, kc, tsls[j]],
                                start=(kc == 0),
                                stop=(kc == DC - 1),
                            )
                else:
                    for dcp in range(DC // 2):
                        for j in range(2):
                            nc.tensor.matmul(
                                z_ps[j],
                                lhsT=gate_w(dcp, dc),
                                rhs=x8_s[:, 2 * dcp : 2 * dcp + 2, tsls[j]],
                                start=(dcp == 0),
                                stop=(dcp == DC // 2 - 1),
                                perf_mode=pm,
                            )
                gs = []
                for j in range(2):
                    g = work.tile([128, T_TILE], F32, tag="g")
                    nc.scalar.activation(
                        out=g, in_=z_ps[j], func=AF.Sigmoid,
                        scale=(1.0 if GATE_BF16 else 1.0 / 32.0),
                        bias=gb_s[:, dc : dc + 1],
                    )
                    gs.append(g)
                r_ps = [mm_ps.tile([128, T_TILE], F32, tag="mm", name=f"rps{j}")
                        for j in range(2)]
                for wcp in range(WC // 2):
                    for j in range(2):
                        nc.tensor.matmul(
                            r_ps[j],
                            lhsT=retr_w(wcp, dc),
                            rhs=at8_s[:, 2 * wcp : 2 * wcp + 2, tsls[j]],
                            start=(wcp == 0),
                            stop=(wcp == WC // 2 - 1),
                            perf_mode=pm,
                        )
                for j in range(2):
                    # out = r/64 + g*(xc - r/64), via:
                    #   t = r/64 - xc ; u = g*t ; o = r/64 - u
                    t = work.tile([128, T_TILE], F32, tag="t")
                    nc.vector.scalar_tensor_tensor(
                        out=t, in0=r_ps[j], scalar=1.0 / 64.0,
                        in1=xc_s[:, dc, tsls[j]],
                        op0=ALU.mult, op1=ALU.subtract,
                    )
                    u = work.tile([128, T_TILE], F32, tag="u")
                    nc.vector.tensor_mul(u, t, gs[j])
                    o = work.tile([128, T_TILE], BF16, tag="o")
                    nc.vector.scalar_tensor_tensor(
                        out=o, in0=r_ps[j], scalar=1.0 / 64.0, in1=u,
                        op0=ALU.mult, op1=ALU.subtract,
                    )
                    nc.sync.dma_start(out=outv[:, dc, tsls[j]], in_=o)


_NC_CACHE = None


def _build_nc(reps: int = 1):
    global _NC_CACHE
    if reps == 1 and _NC_CACHE is not None:
        return _NC_CACHE
    nc = bacc.Bacc("TRN2", target_bir_lowering=False, debug=False,
                   enable_asserts=False)
    with tile.TileContext(nc) as tc:
        _body(tc, reps)
    nc.compile()
    if reps == 1:
        _NC_CACHE = nc
    return nc


def _interleave(w0, w1):
    """SwInterleave weight layout: out[p, 2c+i] = w_i[p, 127-c]."""
    out = np.empty((w0.shape[0], 256), dtype=w0.dtype)
    out[:, 0::2] = w0[:, ::-1]
    out[:, 1::2] = w1[:, ::-1]
    return out


def make_in_maps(x, memory_bank, gate_w, gate_b):
    x = np.ascontiguousarray(np.asarray(x, np.float32)).reshape(B * S, D)
    mb8 = np.asarray(memory_bank, np.float32).astype(E4NP)       # [W, D]
    gw8 = (np.asarray(gate_w, np.float32) * 32.0).astype(E4NP)   # [E, D]
    gb_n = np.ascontiguousarray(np.asarray(gate_b, np.float32))
    ones_n = np.full((128, 1), 1.0 / 64.0, E4NP)

    if PERF == "dr":
        mbt_n = np.ascontiguousarray(mb8.T)                       # [D, W]
        mbw_n = np.ascontiguousarray(mb8)                         # [W, D]
    else:
        # mbt[p, dcp, wc, 2c+i] = mb8[wc*128 + 127-c, (2dcp+i)*128 + p]
        mbt_n = np.empty((128, DC // 2, WC, 256), E4NP)
        mbw_n = np.empty((128, WC // 2, DC, 256), E4NP)
        for dcp in range(DC // 2):
            for wc in range(WC):
                w0 = mb8[wc * 128 : wc * 128 + 128, (2 * dcp) * 128 : (2 * dcp + 1) * 128].T
                w1 = mb8[wc * 128 : wc * 128 + 128, (2 * dcp + 1) * 128 : (2 * dcp + 2) * 128].T
                mbt_n[:, dcp, wc, :] = _interleave(w0, w1)
        # mbw[p, wcp, dc, 2c+i] = mb8[(2wcp+i)*128 + p, dc*128 + 127-c]
        for wcp in range(WC // 2):
            for dc in range(DC):
                w0 = mb8[(2 * wcp) * 128 : (2 * wcp + 1) * 128, dc * 128 : dc * 128 + 128]
                w1 = mb8[(2 * wcp + 1) * 128 : (2 * wcp + 2) * 128, dc * 128 : dc * 128 + 128]
                mbw_n[:, wcp, dc, :] = _interleave(w0, w1)

    if GATE_BF16:
        gwt_n = np.ascontiguousarray(np.asarray(gate_w, np.float32).T).astype(BFNP)
    elif PERF == "dr":
        gwt_n = np.ascontiguousarray(gw8.T)                       # [D, E]
    else:
        # gwt[p, dcp, ec, 2c+i] = gw8[ec*128 + 127-c, (2dcp+i)*128 + p]
        gwt_n = np.empty((128, DC // 2, DC, 256), E4NP)
        for dcp in range(DC // 2):
            for ec in range(DC):
                w0 = gw8[ec * 128 : ec * 128 + 128, (2 * dcp) * 128 : (2 * dcp + 1) * 128].T
                w1 = gw8[ec * 128 : ec * 128 + 128, (2 * dcp + 1) * 128 : (2 * dcp + 2) * 128].T
                gwt_n[:, dcp, ec, :] = _interleave(w0, w1)

    in_maps = []
    for c in range(N_CORES):
        xs = x[c * T_CORE : (c + 1) * T_CORE]                     # [T, D]
        xsT = np.ascontiguousarray(xs.T)                          # [D, T]
        in_maps.append(
            {
                "x8": xsT.astype(E4NP),
                "xc": xsT.astype(BFNP),
                "gb": gb_n,
                "ones": ones_n,
                "mbt": mbt_n,
                "mbw": mbw_n,
                "gwt": gwt_n,
            }
        )
    return in_maps


def assemble_out(results):
    shards = [results[c]["outt"].astype(np.float32).T for c in range(N_CORES)]
    return np.concatenate(shards, axis=0).reshape(B, S, D)


def kernel(x, memory_bank, gate_w, gate_b, _run_kwargs=None):
    nc = _build_nc()
    in_maps = make_in_maps(x, memory_bank, gate_w, gate_b)
    res = bass_utils.run_bass_kernel_spmd(
        nc, in_maps, core_ids=list(range(N_CORES)), **(_run_kwargs or {})
    )
    out = assemble_out(res.results)
    if _run_kwargs:
        kernel.last_result = res
    return out


# revision 6
# speedup vs baseline: 1.1725x; 1.0158x over previous
"""ContextRetentionLayer Trainium2 kernel — fp8 DoubleRow version.

Reference computation (per token t, d=1024, W=512 memory slots):
    s[t, w]   = (x[t] . mb[w]) / 32
    attn[t]   = softmax_w(s[t])
    r[t]      = sum_w attn[t, w] * mb[w]
    g[t]      = sigmoid(x[t] @ gw.T + gb)
    out[t]    = g[t] * x[t] + (1 - g[t]) * r[t]

Sharding: 4x4096 = 16384 tokens split evenly across 8 cores (2048 each);
memory_bank / gate weights replicated.

All three big matmuls (scores, retrieved, gate) run in fp8 e4m3 with
perf_mode=DoubleRow: operands live as [128, KC, free] with K-chunk pairs
contracted 256 at a time at ~2x the bf16/f32r column rate.  Tolerance is
2e-2; CPU sim of this exact quantization measures rel err ~1.8e-2
(gate-path dominated; see sim_fp8.py).

Numerics:
  x8 = e4m3(x); mb8 = e4m3(mb); gw8 = e4m3(32*gw)
  s_ps[w,t] = sum_d mb8T . x8          (PSUM f32, = 32*s approx)
  at8       = e4m3(exp(s_ps/32))
  den_ps    = sum_w at8/64             (ones lhsT = 1/64)
  rb        = bcast(1/den_ps) = 64/den
  at8      *= rb                       (in place -> 64*attn, e4m3; keeps
                                        attn out of e4m3 subnormals)
  r_ps      = sum_w mb8 . at8          (= 64*r)
  z_ps      = sum_d gw8T . x8          (= 32*z approx)
  g         = sigmoid(z_ps/32 + gb)
  out       = bf16( r_ps/64 + g*(xc - r_ps/64) ),  xc = bf16(x)
Combine is 3 fused DVE ops (scalar_tensor_tensor folds the 1/64).

Softmax max-subtraction is skipped: scores/32 ~ N(0,1), exp <= ~110 < 240
(e4m3 max), no overflow.  Weight-stationary loop order (weights reused
across 2-4 token tiles per LDWEIGHTS) since DoubleRow disables FWL.
"""

import numpy as np
import ml_dtypes

import concourse.bass as bass
import concourse.tile as tile
from concourse import bacc, bass_utils, mybir
from concourse.bass import ts

AF = mybir.ActivationFunctionType
ALU = mybir.AluOpType
F32 = mybir.dt.float32
BF16 = mybir.dt.bfloat16
F8E4 = mybir.dt.float8e4
E4NP = ml_dtypes.float8_e4m3
BFNP = ml_dtypes.bfloat16

N_CORES = 8
B, S, D = 4, 4096, 1024
W = 512
T_CORE = (B * S) // N_CORES  # 2048 tokens per core
T_TILE = 512                 # moving free dim per matmul (DR pairs: rhs 1024)
NT = T_CORE // T_TILE        # 4 token tiles
DC = D // 128                # 8 chunks of the embed dim
WC = W // 128                # 4 chunks of the memory window

# "dr": natural [128, KC, M] weights, perf_mode=DoubleRow
# "dri": host-interleaved weights, perf_mode=DoubleRowSwInterleave
PERF = "dri"
GATE_BF16 = False            # fallback: gate matmul in bf16 (better precision)

_PM = {
    "dr": mybir.MatmulPerfMode.DoubleRow,
    "dri": mybir.MatmulPerfMode.DoubleRowSwInterleave,
}


def _body(tc: tile.TileContext, reps: int = 1):
    nc = tc.nc

    x8 = nc.dram_tensor("x8", (D, T_CORE), F8E4, kind="ExternalInput").ap()
    xc = nc.dram_tensor("xc", (D, T_CORE), BF16, kind="ExternalInput").ap()
    gb = nc.dram_tensor("gb", (D,), F32, kind="ExternalInput").ap()
    ones = nc.dram_tensor("ones", (128, 1), F8E4, kind="ExternalInput").ap()
    if PERF == "dr":
        mbt = nc.dram_tensor("mbt", (D, W), F8E4, kind="ExternalInput").ap()
        mbw = nc.dram_tensor("mbw", (W, D), F8E4, kind="ExternalInput").ap()
    else:
        mbt = nc.dram_tensor("mbt", (128, DC // 2, WC, 256), F8E4,
                             kind="ExternalInput").ap()
        mbw = nc.dram_tensor("mbw", (128, WC // 2, DC, 256), F8E4,
                             kind="ExternalInput").ap()
    if GATE_BF16:
        gwt = nc.dram_tensor("gwt", (D, D), BF16, kind="ExternalInput").ap()
    elif PERF == "dr":
        gwt = nc.dram_tensor("gwt", (D, D), F8E4, kind="ExternalInput").ap()
    else:
        gwt = nc.dram_tensor("gwt", (128, DC // 2, DC, 256), F8E4,
                             kind="ExternalInput").ap()
    outT = nc.dram_tensor("outt", (D, T_CORE), BF16, kind="ExternalOutput").ap()

    for _rep in range(reps):
        _emit_once(tc, x8, xc, gb, ones, mbt, mbw, gwt, outT)


def _emit_once(tc, x8, xc, gb, ones, mbt, mbw, gwt, outT):
    nc = tc.nc
    pm = _PM[PERF]
    with (
        tc.tile_pool(name="const", bufs=1) as const,
        tc.tile_pool(name="big", bufs=1) as big,
        tc.tile_pool(name="work", bufs=3) as work,
        tc.tile_pool(name="mm_ps", bufs=7, space="PSUM") as mm_ps,
        tc.tile_pool(name="den_psp", bufs=1, space="PSUM") as den_psp,
    ):
        # ---- SBUF tiles
        if PERF == "dr":
            mbt_s = const.tile([128, DC, W], F8E4)
            mbw_s = const.tile([128, WC, D], F8E4)
        else:
            mbt_s = const.tile([128, DC // 2, WC, 256], F8E4)
            mbw_s = const.tile([128, WC // 2, DC, 256], F8E4)
        if GATE_BF16:
            gwt_s = const.tile([128, DC, D], BF16)
        elif PERF == "dr":
            gwt_s = const.tile([128, DC, D], F8E4)
        else:
            gwt_s = const.tile([128, DC // 2, DC, 256], F8E4)
        gb_s = const.tile([128, DC], F32)
        ones_s = const.tile([128, 1], F8E4)

        x8_s = big.tile([128, DC, T_CORE], F8E4)
        xc_s = big.tile([128, DC, T_CORE], BF16)
        at8_s = big.tile([128, WC, T_CORE], F8E4)
        rd_s = big.tile([1, T_CORE], F32)
        rb_s = big.tile([128, T_CORE], F32)

        x8v = x8.rearrange("(c p) t -> p c t", p=128)
        xcv = xc.rearrange("(c p) t -> p c t", p=128)
        outv = outT.rearrange("(c p) t -> p c t", p=128)

        # ---- need-ordered loads: scores weights + x tile 0 first.
        if PERF == "dr":
            mbtv = mbt.rearrange("(c p) w -> p c w", p=128)
            for c in range(DC):
                nc.sync.dma_start(out=mbt_s[:, c, :], in_=mbtv[:, c, :])
        else:
            nc.sync.dma_start(out=mbt_s, in_=mbt)
        nc.sync.dma_start(out=x8_s[:, :, ts(0, T_TILE)], in_=x8v[:, :, ts(0, T_TILE)])
        nc.sync.dma_start(out=ones_s, in_=ones)
        nc.sync.dma_start(out=gb_s, in_=gb.rearrange("(c p) -> p c", p=128))
        for ti in range(1, NT):
            nc.sync.dma_start(
                out=x8_s[:, :, ts(ti, T_TILE)], in_=x8v[:, :, ts(ti, T_TILE)]
            )
        if PERF == "dr":
            mbwv = mbw.rearrange("(c p) d -> p c d", p=128)
            for c in range(WC):
                nc.sync.dma_start(out=mbw_s[:, c, :], in_=mbwv[:, c, :])
        else:
            nc.sync.dma_start(out=mbw_s, in_=mbw)
        if GATE_BF16 or PERF == "dr":
            gwtv = gwt.rearrange("(c p) e -> p c e", p=128)
            for c in range(DC):
                nc.sync.dma_start(out=gwt_s[:, c, :], in_=gwtv[:, c, :])
        else:
            nc.sync.dma_start(out=gwt_s, in_=gwt)
        for ti in range(NT):
            nc.sync.dma_start(
                out=xc_s[:, :, ts(ti, T_TILE)], in_=xcv[:, :, ts(ti, T_TILE)]
            )

        def score_w(dcp, wc):
            if PERF == "dr":
                return mbt_s[:, 2 * dcp : 2 * dcp + 2, ts(wc, 128)]
            return mbt_s[:, dcp, wc, :]

        def retr_w(wcp, dc):
            if PERF == "dr":
                return mbw_s[:, 2 * wcp : 2 * wcp + 2, ts(dc, 128)]
            return mbw_s[:, wcp, dc, :]

        def gate_w(dcp, ec):
            if PERF == "dr":
                return gwt_s[:, 2 * dcp : 2 * dcp + 2, ts(ec, 128)]
            return gwt_s[:, dcp, ec, :]

        # ---- pass 1: scores + exp.  Weight-stationary: each (wc, dcp)
        # weight pair streams all 4 token tiles before switching.
        for wc in range(WC):
            s_ps = [mm_ps.tile([128, T_TILE], F32, tag="mm", name=f"sps{wc}_{i}")
                    for i in range(NT)]
            for dcp in range(DC // 2):
                for ti in range(NT):
                    nc.tensor.matmul(
                        s_ps[ti],
                        lhsT=score_w(dcp, wc),
                        rhs=x8_s[:, 2 * dcp : 2 * dcp + 2, ts(ti, T_TILE)],
                        start=(dcp == 0),
                        stop=(dcp == DC // 2 - 1),
                        perf_mode=pm,
                    )
            for ti in range(NT):
                nc.scalar.activation(
                    out=at8_s[:, wc, ts(ti, T_TILE)], in_=s_ps[ti],
                    func=AF.Exp, scale=1.0 / 32.0,
                )

        # ---- denominators + normalize (at8 <- 64*attn, in place)
        for ti in range(NT):
            tsl = ts(ti, T_TILE)
            den_ps = den_psp.tile([1, T_TILE], F32, tag="den")
            for wc in range(WC):
                nc.tensor.matmul(
                    den_ps,
                    lhsT=ones_s,
                    rhs=at8_s[:, wc, tsl],
                    start=(wc == 0),
                    stop=(wc == WC - 1),
                )
            rscr = work.tile([1, T_TILE], F32, tag="rscr")
            nc.vector.reciprocal_approx_accurate(
                out=rd_s[:, tsl], in_=den_ps, scratch=rscr
            )
            nc.gpsimd.partition_broadcast(rb_s[:, tsl], rd_s[:, tsl])
            for wc in range(WC):
                nc.vector.tensor_mul(at8_s[:, wc, tsl], at8_s[:, wc, tsl], rb_s[:, tsl])

        # ---- pass 2: gate + retrieved + combine, in token-tile pairs so at
        # most ~4 PSUM banks per (tib, dc) group and weights are reused x2.
        for tib in range(NT // 2):
            tis = (2 * tib, 2 * tib + 1)
            tsls = [ts(t, T_TILE) for t in tis]
            for dc in range(DC):
                z_ps = [mm_ps.tile([128, T_TILE], F32, tag="mm", name=f"zps{j}")
                        for j in range(2)]
                if GATE_BF16:
                    for kc in range(DC):
                        for j in range(2):
                            nc.tensor.matmul(
                                z_ps[j],
                                lhsT=gwt_s[:, kc, ts(dc, 128)],
                                rhs=xc_s[:, kc, tsls[j]],
                                start=(kc == 0),
                                stop=(kc == DC - 1),
                            )
                else:
                    for dcp in range(DC // 2):
                        for j in range(2):
                            nc.tensor.matmul(
                                z_ps[j],
                                lhsT=gate_w(dcp, dc),
                                rhs=x8_s[:, 2 * dcp : 2 * dcp + 2, tsls[j]],
                                start=(dcp == 0),
                                stop=(dcp == DC // 2 - 1),
                                perf_mode=pm,
                            )
                gs = []
                for j in range(2):
                    g = work.tile([128, T_TILE], F32, tag="g")
                    nc.scalar.activation(
                        out=g, in_=z_ps[j], func=AF.Sigmoid,
                        scale=(1.0 if GATE_BF16 else 1.0 / 32.0),
                        bias=gb_s[:, dc : dc + 1],
                    )
                    gs.append(g)
                r_ps = [mm_ps.tile([128, T_TILE], F32, tag="mm", name=f"rps{j}")
                        for j in range(2)]
                for wcp in range(WC // 2):
                    for j in range(2):
                        nc.tensor.matmul(
                            r_ps[j],
                            lhsT=retr_w(wcp, dc),
                            rhs=at8_s[:, 2 * wcp : 2 * wcp + 2, tsls[j]],
                            start=(wcp == 0),
                            stop=(wcp == WC // 2 - 1),
                            perf_mode=pm,
                        )
                for j in range(2):
                    # out = r/64 + g*(xc - r/64), via:
                    #   t = r/64 - xc ; u = g*t ; o = r/64 - u
                    t = work.tile([128, T_TILE], F32, tag="t")
                    nc.vector.scalar_tensor_tensor(
                        out=t, in0=r_ps[j], scalar=1.0 / 64.0,
                        in1=xc_s[:, dc, tsls[j]],
                        op0=ALU.mult, op1=ALU.subtract,
                    )
                    u = work.tile([128, T_TILE], F32, tag="u")
                    nc.vector.tensor_mul(u, t, gs[j])
                    o = work.tile([128, T_TILE], BF16, tag="o")
                    nc.vector.scalar_tensor_tensor(
                        out=o, in0=r_ps[j], scalar=1.0 / 64.0, in1=u,
                        op0=ALU.mult, op1=ALU.subtract,
                    )
                    nc.sync.dma_start(out=outv[:, dc, tsls[j]], in_=o)


_NC_CACHE = None


def _build_nc(reps: int = 1):
    global _NC_CACHE
    if reps == 1 and _NC_CACHE is not None:
        return _NC_CACHE
    nc = bacc.Bacc("TRN2", target_bir_lowering=False, debug=False,
                   enable_asserts=False)
    with tile.TileContext(nc) as tc:
        _body(tc, reps)
    nc.compile()
    if reps == 1:
        _NC_CACHE = nc
    return nc


def _interleave(w0, w1):
    """SwInterleave weight layout: out[p, 2c+i] = w_i[p, 127-c]."""
    out = np.empty((w0.shape[0], 256), dtype=w0.dtype)
    out[:, 0::2] = w0[:, ::-1]
    out[:, 1::2] = w1[:, ::-1]
    return out


def make_in_maps(x, memory_bank, gate_w, gate_b):
    x = np.ascontiguousarray(np.asarray(x, np.float32)).reshape(B * S, D)
    mb8 = np.asarray(memory_bank, np.float32).astype(E4NP)       # [W, D]
    gw8 = (np.asarray(gate_w, np.float32) * 32.0).astype(E4NP)   # [E, D]
    gb_n = np.ascontiguousarray(np.asarray(gate_b, np.float32))
    ones_n = np.full((128, 1), 1.0 / 64.0, E4NP)

    if PERF == "dr":
        mbt_n = np.ascontiguousarray(mb8.T)                       # [D, W]
        mbw_n = np.ascontiguousarray(mb8)                         # [W, D]
    else:
        # mbt[p, dcp, wc, 2c+i] = mb8[wc*128 + 127-c, (2dcp+i)*128 + p]
        mbt_n = np.empty((128, DC // 2, WC, 256), E4NP)
        mbw_n = np.empty((128, WC // 2, DC, 256), E4NP)
        for dcp in range(DC // 2):
            for wc in range(WC):
                w0 = mb8[wc * 128 : wc * 128 + 128, (2 * dcp) * 128 : (2 * dcp + 1) * 128].T
                w1 = mb8[wc * 128 : wc * 128 + 128, (2 * dcp + 1) * 128 : (2 * dcp + 2) * 128].T
                mbt_n[:, dcp, wc, :] = _interleave(w0, w1)
        # mbw[p, wcp, dc, 2c+i] = mb8[(2wcp+i)*128 + p, dc*128 + 127-c]
        for wcp in range(WC // 2):
            for dc in range(DC):
                w0 = mb8[(2 * wcp) * 128 : (2 * wcp + 1) * 128, dc * 128 : dc * 128 + 128]
                w1 = mb8[(2 * wcp + 1) * 128 : (2 * wcp + 2) * 128, dc * 128 : dc * 128 + 128]
                mbw_n[:, wcp, dc, :] = _interleave(w0, w1)

    if GATE_BF16:
        gwt_n = np.ascontiguousarray(np.asarray(gate_w, np.float32).T).astype(BFNP)
    elif PERF == "dr":
        gwt_n = np.ascontiguousarray(gw8.T)                       # [D, E]
    else:
        # gwt[p, dcp, ec, 2c+i] = gw8[ec*128 + 127-c, (2dcp+i)*128 + p]
        gwt_n = np.empty((128, DC // 2, DC, 256), E4NP)
        for dcp in range(DC // 2):
            for ec in range(DC):
                w0 = gw8[ec * 128 : ec * 128 + 128, (2 * dcp) * 128 : (2 * dcp + 1) * 128].T
                w1 = gw8[ec * 128 : ec * 128 + 128, (2 * dcp + 1) * 128 : (2 * dcp + 2) * 128].T
                gwt_n[:, dcp, ec, :] = _interleave(w0, w1)

    in_maps = []
    for c in range(N_CORES):
        xs = x[c * T_CORE : (c + 1) * T_CORE]                     # [T, D]
        xsT = np.ascontiguousarray(xs.T)                          # [D, T]
        in_maps.append(
            {
                "x8": xsT.astype(E4NP),
                "xc": xsT.astype(BFNP),
                "gb": gb_n,
                "ones": ones_n,
                "mbt": mbt_n,
                "mbw": mbw_n,
                "gwt": gwt_n,
            }
        )
    return in_maps


def assemble_out(results):
    shards = [results[c]["outt"].astype(np.float32).T for c in range(N_CORES)]
    return np.concatenate(shards, axis=0).reshape(B, S, D)


def kernel(x, memory_bank, gate_w, gate_b, _run_kwargs=None):
    nc = _build_nc()
    in_maps = make_in_maps(x, memory_bank, gate_w, gate_b)
    res = bass_utils.run_bass_kernel_spmd(
        nc, in_maps, core_ids=list(range(N_CORES)), **(_run_kwargs or {})
    )
    out = assemble_out(res.results)
    if _run_kwargs:
        kernel.last_result = res
    return out
